# revision 1
# baseline (speedup 1.0000x reference)
"""Trainium2 Bass kernel for sparse-attention 3D-ViT (nn_BaseModel_44341242364529).

Sharding: sequence axis L across 8 cores (512 patch rows each + replicated
BOS/EOS tile).  Design:
- all matmuls bf16 (4x PE throughput vs fp32), fp32 PSUM accumulation
- attention bias is MULTIPLICATIVE: P = exp(scale*S) * expB with expB
  preloaded once into SBUF as bf16 (masked slots = 0) -> no bias DMA, no
  PSUM bias adds
- layer 0 needs NO collective: each core redundantly computes k/v for its
  1536-row band (plus the two corner tiles EOS attends) from the input
  patches directly
- layer 1 uses a single fused bf16 AllGather of (kT chunk, v chunk in
  AV-ready [tile, head, dh+1] layout); local-key attention runs overlap it
- attention is w-grouped: one S matmul per (head, key-tile run) with the
  kT tile stationary, exp/bias-mult per <=1024-col group, transposed AV
  with the 65-col v tile stationary, per-head wo with the softmax
  denominator folded in after wo (den row extracted by a 16-partition DMA
  transpose)
- all transposes via DMA-engine xbar transpose (dma_start_transpose)
- LN per-tile (stats -> sqrt+recip rstd -> gpsimd apply -> transposes) so
  downstream matmuls start before the last tile
"""

import numpy as np
import ml_dtypes

# model dims (hardcoded per spec)
IMG, PATCH, D, H, NLAYERS, DFF = 64, 4, 256, 4, 2, 1024
GT = IMG // PATCH          # 16
N = GT * GT * GT           # 4096
L = N + 2                  # 4098
DH = D // H                # 64
PVOL = PATCH ** 3          # 64
NCORES = 8
LC = 512                   # real patch rows per core
LLOC = 640                 # padded local rows (5 tiles of 128)
NT = 5                     # local row tiles
NBT = 12                   # band key tiles (1536 rows)
SCALE = 1.0 / np.sqrt(DH)  # 0.125
KCH = 2 * LC               # kT chunk cols (2 dh-blocks x 512)
VCH = 4 * H * (DH + 1)     # v chunk cols (4 tiles x H x 65)
BF16 = ml_dtypes.bfloat16


def _kts_for_tile(t):
    if t < 4:
        return [("win", t), ("win", t + 3), ("win", t + 4), ("win", t + 5),
                ("win", t + 6), ("win", t + 7), ("win", t + 8), ("loc4", 0)]
    return [("loc4", 0), ("x", 0), ("x", 1)]


# w-grouped attention runs for query tiles 0..3: (w, tlist, av_start, av_stop).
# One S matmul (stationary kT_w) + exp + bias-mult + one AV matmul
# (stationary v_w) per run.  LOCAL key tiles (loc4, w>=8 = own chunk) come
# first so in layer 1 they overlap the AllGather; the loc4 run opens every
# AV region (start) and w7 — last writer of all four regions — closes them.
RUNS = [("loc4", (0, 1, 2, 3), True, False),
        (8, (0, 1, 2, 3), False, False), (9, (1, 2, 3), False, False),
        (10, (2, 3), False, False), (11, (3,), False, False),
        (0, (0,), False, False), (1, (1,), False, False), (2, (2,), False, False),
        (3, (0,), False, False), (3, (3,), False, False),
        (4, (0, 1), False, False), (5, (0, 1, 2), False, False),
        (6, (0, 1, 2, 3), False, False), (7, (0, 1, 2, 3), False, True)]
RUNCOLS = []
_off = 0
for _w, _ts, _a, _b in RUNS:
    RUNCOLS.append(_off)
    _off += len(_ts) * 128
EXPW_COLS = _off  # 4096

# run groups for coarse exp/mult: contiguous run ranges, <=1024 cols each
# (PSUM tile of 2 banks).  Forced break after run 4 keeps local-only groups
# separate from gather-dependent ones so their exp/AV never block on the
# collective.
GROUPS = []
_g0 = 0
for _ri in range(len(RUNS) + 1):
    if (_ri == len(RUNS) or _ri == 5
            or (RUNCOLS[_ri] - RUNCOLS[_g0]) + len(RUNS[_ri][1]) * 128 > 1024):
        GROUPS.append((_g0, _ri, RUNCOLS[_g0],
                       (RUNCOLS[_ri - 1] + len(RUNS[_ri - 1][1]) * 128) - RUNCOLS[_g0]))
        _g0 = _ri
        if _ri == len(RUNS):
            break


_prog_cache = {}


def _build_program():
    import concourse.bass as bass
    import concourse.bacc as bacc
    import concourse.tile as tile
    from concourse import mybir

    f32 = mybir.dt.float32
    bf16 = mybir.dt.bfloat16
    AF = mybir.ActivationFunctionType
    nc = bacc.Bacc("TRN2", target_bir_lowering=False, debug=False,
                   num_devices=NCORES)

    def din(name, shape, dt_=bf16):
        return nc.declare_dram_parameter(name, list(shape), dt_, isOutput=False)

    imgT_d = din("imgT", [PVOL, NBT * 128])
    imgTx_d = din("imgTx", [PVOL, 256])
    emb_d = din("emb", [LLOC, D], f32)
    wq_d = din("wq", [NLAYERS, D, D])
    wk_d = din("wk", [NLAYERS, D, D])
    wv_d = din("wv", [NLAYERS, D, D])
    wo_d = din("wo", [NLAYERS, D, D])
    w1_d = din("w1", [NLAYERS, D, DFF])
    w2_d = din("w2", [NLAYERS, DFF, D])
    pw_d = din("patch_w", [PVOL, D])
    expW_d = din("expW", [H, 128, EXPW_COLS])
    expB4_d = din("expB4", [H, 128, 3 * 128])
    out_d = nc.declare_dram_parameter("out", [LLOC, D], f32, isOutput=True)

    # internal DRAM for the single layer-1 collective
    cc_in = nc.dram_tensor("cc_in", [128, KCH + VCH], bf16)
    cc_gat = nc.dram_tensor("cc_gat", [NCORES + 2, 128, KCH + VCH], bf16,
                            addr_space="Shared")

    from contextlib import ExitStack
    with tile.TileContext(nc) as tc, ExitStack() as ctx:
        sing = ctx.enter_context(tc.tile_pool(name="sing", bufs=1))
        wk_pool = ctx.enter_context(tc.tile_pool(name="wrk", bufs=1))
        wk2_pool = ctx.enter_context(tc.tile_pool(name="wrk2", bufs=2))
        pe_pool = ctx.enter_context(tc.tile_pool(name="pexp", bufs=4))
        pm_pool = ctx.enter_context(tc.tile_pool(name="pmul", bufs=2))
        ps_s = ctx.enter_context(tc.tile_pool(name="pss", bufs=2, space="PSUM"))
        ps_aoT = ctx.enter_context(tc.tile_pool(name="pst", bufs=2, space="PSUM"))
        ps_pr = ctx.enter_context(tc.tile_pool(name="psp", bufs=2, space="PSUM"))

        sync = nc.sync
        pid = sync.partition_id()

        # ---------------- constants / weights ----------------
        # load order matters: the DMA queue drains serially, so tensors are
        # requested in first-use order (patch embed -> layer-0 k/v -> ...)
        pw = sing.tile([PVOL, D], bf16, tag="pw")
        sync.dma_start(out=pw[:], in_=pw_d[:, :])
        imgT = sing.tile([PVOL, NBT * 128], bf16, tag="imgT")
        sync.dma_start(out=imgT[:], in_=imgT_d[:, :])
        emb = sing.tile([128, NT, D], f32, tag="emb")
        sync.dma_start(out=emb[:], in_=emb_d.rearrange("(t p) d -> p t d", p=128))
        imgTx = sing.tile([PVOL, 256], bf16, tag="imgTx")
        sync.dma_start(out=imgTx[:], in_=imgTx_d[:, :])

        W = {}

        def loadw(nm, dt_, l):
            kd = 8 if nm == "w2" else 2
            t_ = sing.tile([128, kd, dt_.shape[2]], bf16, tag=f"{nm}{l}")
            sync.dma_start(out=t_[:], in_=dt_[l].rearrange("(k p) n -> p k n", p=128))
            W[(nm, l)] = t_

        def loadwoh(l):
            t_ = sing.tile([64, H, D], bf16, tag=f"woh{l}")
            sync.dma_start(out=t_[:], in_=wo_d[l].rearrange("(h p) n -> p h n", p=64))
            W[("woh", l)] = t_

        for nm, dt_ in (("wk", wk_d), ("wv", wv_d), ("wq", wq_d), ("wo", wo_d)):
            loadw(nm, dt_, 0)
        loadwoh(0)
        # heavier constants (expW 32KB/part, FFN + layer-1 weights) are
        # loaded lazily inside layer 0 — after the LN1 transposes are in the
        # DMA queue — so the startup critical path isn't stuck behind them
        expW = sing.tile([128, H, EXPW_COLS], bf16, tag="expW")
        expB4 = sing.tile([128, H, 3 * 128], bf16, tag="expB4")
        zt = sing.tile([128, KCH + VCH], bf16, tag="zero")

        def load_rest():
            for nm, dt_ in (("w1", w1_d), ("w2", w2_d)):
                loadw(nm, dt_, 0)
            for nm, dt_ in (("wk", wk_d), ("wv", wv_d), ("wq", wq_d),
                            ("wo", wo_d), ("w1", w1_d), ("w2", w2_d)):
                loadw(nm, dt_, 1)
            loadwoh(1)
            # zero the 2 pad slots of the gathered buffer
            nc.vector.memset(zt[:], 0.0)
            for ch in range(2):
                sync.dma_start(out=cc_gat[ch], in_=zt[:])

        eps_sb = sing.tile([128, 1], f32, tag="eps")
        nc.vector.memset(eps_sb[:], 1e-5)

        # ---------------- persistent activations ----------------
        x_sb = wk_pool.tile([128, NT, D], f32, tag="x")
        xb_sb = wk_pool.tile([128, 8, D], f32, tag="xb")     # band tiles 0..7
        xx_sb = wk_pool.tile([128, 2, D], f32, tag="xx")     # corner tiles 29,31
        h_sb = wk_pool.tile([128, NT, D], bf16, tag="h")
        hb_sb = wk_pool.tile([128, 8, D], bf16, tag="hb")
        hx_sb = wk_pool.tile([128, 2, D], bf16, tag="hx")
        hT = wk_pool.tile([128, 2, NT * 128], bf16, tag="hT")
        hTb = wk_pool.tile([128, 2, 8 * 128], bf16, tag="hTb")
        hTx = wk_pool.tile([128, 2, 256], bf16, tag="hTx")
        qT = wk_pool.tile([128, 2, LLOC], bf16, tag="qT")
        kT_win = wk_pool.tile([128, 2, 8 * 128], bf16, tag="kwin")
        kT_loc = wk_pool.tile([128, 2, 4 * 128], bf16, tag="kloc")
        kT4 = wk_pool.tile([128, 2, 128], bf16, tag="kT4")
        kTx = wk_pool.tile([128, 2, 256], bf16, tag="kTx")
        v_win = wk_pool.tile([128, 8, H, DH + 1], bf16, tag="vwin")
        v_loc = wk_pool.tile([128, 4, H, DH + 1], bf16, tag="vloc")
        v4 = wk_pool.tile([128, H, DH + 1], bf16, tag="v4")
        vx = wk_pool.tile([128, 2, H, DH + 1], bf16, tag="vx")
        yT = wk_pool.tile([128, 8, LLOC], bf16, tag="yT")
        stats = wk_pool.tile([128, 16, 6], f32, tag="stats")
        mv = wk_pool.tile([128, 16, 2], f32, tag="mv")
        lnv = wk_pool.tile([128, 16], f32, tag="lnv")
        rstd = wk_pool.tile([128, 16], f32, tag="rstd")
        o_sb = wk_pool.tile([128, NT, D], f32, tag="osb")
        aoT_sb = wk_pool.tile([128, H, 512], bf16, tag="aoTsb")
        den_all = wk_pool.tile([128, 4, H, 16], bf16, tag="den")
        # rows 64.. of aoT_sb feed the 16-partition den transpose; zero once
        nc.vector.memset(aoT_sb[64:128, :, :], 0.0)

        # ones columns (never overwritten by local v writes; gathered data
        # for layer 1 carries its own ones column)
        nc.vector.memset(v_win[:, :, :, DH:DH + 1], 1.0)
        nc.vector.memset(v_loc[:, :, :, DH:DH + 1], 1.0)
        nc.vector.memset(v4[:, :, DH:DH + 1], 1.0)
        nc.vector.memset(vx[:, :, :, DH:DH + 1], 1.0)

        # ---------------- patch embed ----------------
        for bt in range(NBT):
            ps = ps_pr.tile([128, 512], f32, tag="pr")
            nc.tensor.matmul(ps[:, 0:D], lhsT=imgT[:, bt * 128:(bt + 1) * 128],
                             rhs=pw[:], start=True, stop=True)
            if bt >= 8:
                nc.vector.tensor_add(x_sb[:, bt - 8, :], ps[:, 0:D],
                                     emb[:, bt - 8, :])
            else:
                nc.vector.tensor_add(xb_sb[:, bt, :], ps[:, 0:D], emb[:, 0, :])
        for xi in range(2):
            ps = ps_pr.tile([128, 512], f32, tag="pr")
            nc.tensor.matmul(ps[:, 0:D], lhsT=imgTx[:, xi * 128:(xi + 1) * 128],
                             rhs=pw[:], start=True, stop=True)
            nc.vector.tensor_add(xx_sb[:, xi, :], ps[:, 0:D], emb[:, 0, :])
        nc.vector.tensor_copy(out=x_sb[:, 4, :], in_=emb[:, 4, :])

        # ---------------- helpers ----------------
        def layer_norm(groups, tr=None):
            """groups: list of (src_ap, dst_ap) row-major [128, D] tiles.
            Per-tile chain (stats -> rstd via DVE pow -> apply on gpsimd),
            optionally followed by the tile's hT transposes (tr callback) so
            downstream consumers start before the last tile finishes."""
            n = len(groups)
            assert n <= 16
            for i, (src, dst) in enumerate(groups):
                nc.vector.bn_stats(out=stats[:, i, :], in_=src)
                nc.vector.bn_aggr(out=mv[:, i, :], in_=stats[:, i, :])
                nc.scalar.activation(out=rstd[:, i:i + 1], in_=mv[:, i, 1:2],
                                     func=AF.Sqrt, bias=eps_sb[:], scale=1.0)
                nc.vector.reciprocal(out=rstd[:, i:i + 1], in_=rstd[:, i:i + 1])
                nc.gpsimd.tensor_scalar(out=dst, in0=src,
                                        scalar1=mv[:, i, 0:1],
                                        scalar2=rstd[:, i:i + 1],
                                        op0=mybir.AluOpType.subtract,
                                        op1=mybir.AluOpType.mult)
                if tr is not None:
                    tr(i)

        def transpose_h(src, dst_cols, dst=None):
            """row-major bf16 [128, D] -> feature-major cols [2][dst_cols:+128]"""
            d = hT if dst is None else dst
            for dt_ in range(2):
                sync.dma_start_transpose(
                    out=d[:, dt_, dst_cols:dst_cols + 128],
                    in_=src[:, dt_ * 128:(dt_ + 1) * 128])

        def proj_from(wsb, j, rhs_t, rhs_cols, out_ap, chunk=512):
            """out[128, rhs_cols] = W_j^T @ rhs (contraction over D)"""
            c0 = 0
            while c0 < rhs_cols:
                c1 = min(c0 + chunk, rhs_cols)
                ps = ps_pr.tile([128, 512], f32, tag="pr")
                for i in range(2):
                    nc.tensor.matmul(ps[:, 0:c1 - c0],
                                     lhsT=wsb[:, i, j * 128:(j + 1) * 128],
                                     rhs=rhs_t[:, i, c0:c1],
                                     start=(i == 0), stop=(i == 1))
                nc.vector.tensor_copy(out=out_ap[:, c0:c1], in_=ps[:, 0:c1 - c0])
                c0 = c1

        def v_proj(wsb, lhsT_ap, dst_hx):
            """row-major v tile: dst [128, H, DH] slots <- h-tile @ wv"""
            ps = ps_aoT.tile([128, 512], f32, tag="aoT")
            for i in range(2):
                nc.tensor.matmul(ps[:, 0:D], lhsT=lhsT_ap[i], rhs=wsb[:, i, :],
                                 start=(i == 0), stop=(i == 1))
            nc.scalar.copy(out=dst_hx,
                           in_=ps[:, 0:D].rearrange("p (h x) -> p h x", h=H))

        # ---------------- layers ----------------
        for l in range(NLAYERS):
            wq_sb, wk_sb = W[("wq", l)], W[("wk", l)]
            wv_sb, wo_sb = W[("wv", l)], W[("wo", l)]

            # LN1 (+ band/corner tiles for layer 0)
            groups = [(x_sb[:, t, :], h_sb[:, t, :]) for t in range(NT)]
            if l == 0:
                groups += [(xb_sb[:, b, :], hb_sb[:, b, :]) for b in range(8)]
                groups += [(xx_sb[:, i, :], hx_sb[:, i, :]) for i in range(2)]
            def _tr1(i):
                if i < NT:
                    transpose_h(h_sb[:, i, :], i * 128)
                elif i < NT + 8:
                    transpose_h(hb_sb[:, i - NT, :], (i - NT) * 128, dst=hTb)
                else:
                    transpose_h(hx_sb[:, i - NT - 8, :], (i - NT - 8) * 128,
                                dst=hTx)
            layer_norm(groups, tr=_tr1)

            # v first: each tile needs only its own hT transposes
            for lt in range(NT):
                dst = (v_loc[:, lt, :, 0:DH] if lt < 4
                       else v4[:, :, 0:DH])
                v_proj(wv_sb, [hT[:, i, lt * 128:(lt + 1) * 128] for i in range(2)],
                       dst)
            # k: local (own-chunk tiles + loc4); band/corners follow later
            for j in range(2):
                proj_from(wk_sb, j, hT, 512, kT_loc[:, j, :], chunk=128)
            for j in range(2):
                ps = ps_pr.tile([128, 512], f32, tag="pr")
                for i in range(2):
                    nc.tensor.matmul(ps[:, 0:128],
                                     lhsT=wk_sb[:, i, j * 128:(j + 1) * 128],
                                     rhs=hT[:, i, 512:640],
                                     start=(i == 0), stop=(i == 1))
                nc.vector.tensor_copy(out=kT4[:, j, :], in_=ps[:, 0:128])

            if l == 0:
                sync.dma_start(out=expW[:], in_=expW_d.rearrange("h p x -> p h x"))
                sync.dma_start(out=expB4[:], in_=expB4_d.rearrange("h p x -> p h x"))
            else:
                # push own chunk; the collective itself is issued after
                # phase-A attention below so local S/exp/mult overlap it
                sync.dma_start(out=cc_in[:, 0:KCH].rearrange(
                                   "p (a b) -> p a b", a=2),
                               in_=kT_loc[:, :, :])
                sync.dma_start(out=cc_in[:, KCH:KCH + VCH].rearrange(
                                   "p (a b c) -> p a b c", a=4, b=H),
                               in_=v_loc[:, :, :, :])

            # q (overlaps the gather)
            for j in range(2):
                proj_from(wq_sb, j, hT, LLOC, qT[:, j, :])

            # ---- attention, query tiles 0..3 ----
            # phase A: S/exp/mult for the LOCAL run groups of every head
            # (in layer 1 this overlaps the AllGather); phase B: gathered
            # groups, then all AV matmuls per head.
            woh = W[("woh", l)]

            def s_group(hh, g0, g1, goff, gcols, ph):
                pb, dt_ = (hh % 2) * 64, hh // 2
                st = ps_s.tile([128, 1024], f32, tag="st")
                for ri in range(g0, g1):
                    w, ts, _a, _b = RUNS[ri]
                    ncol = len(ts) * 128
                    q0 = ts[0] * 128
                    lo = RUNCOLS[ri] - goff
                    if w == "loc4":
                        kl = kT4[pb:pb + 64, dt_, :]
                    elif w >= 8:
                        kl = kT_loc[pb:pb + 64, dt_, (w - 8) * 128:(w - 7) * 128]
                    else:
                        kl = kT_win[pb:pb + 64, dt_, w * 128:(w + 1) * 128]
                    # split any matmul whose output would cross the
                    # 512-col PSUM bank boundary
                    cuts = [0, ncol]
                    if lo < 512 < lo + ncol:
                        cuts = [0, 512 - lo, ncol]
                    for a, b in zip(cuts[:-1], cuts[1:]):
                        nc.tensor.matmul(
                            st[:, lo + a:lo + b], lhsT=kl,
                            rhs=qT[pb:pb + 64, dt_, q0 + a:q0 + b],
                            start=True, stop=True)
                pe = pm_pool.tile([128, 1024], bf16, tag="pe")
                nc.scalar.activation(out=pe[:, 0:gcols], in_=st[:, 0:gcols],
                                     func=AF.Exp, scale=float(SCALE))
                nc.vector.tensor_mul(ph[:, goff:goff + gcols],
                                     pe[:, 0:gcols],
                                     expW[:, hh, goff:goff + gcols])

            phs = []
            for hh in range(H):
                ph = pe_pool.tile([128, EXPW_COLS], bf16, tag="ph")
                phs.append(ph)
                for g0, g1, goff, gcols in GROUPS[:2]:
                    s_group(hh, g0, g1, goff, gcols, ph)

            if l == 0:
                # band k/v computed locally (no collective); overlaps
                # phase-A attention on the other engines
                for j in range(2):
                    proj_from(wk_sb, j, hTb, 1024, kT_win[:, j, 0:8 * 128])
                    proj_from(wk_sb, j, hTx, 256, kTx[:, j, :])
                for b in range(8):
                    v_proj(wv_sb, [hTb[:, i, b * 128:(b + 1) * 128] for i in range(2)],
                           v_win[:, b, :, 0:DH])
                for xi in range(2):
                    v_proj(wv_sb, [hTx[:, i, xi * 128:(xi + 1) * 128] for i in range(2)],
                           vx[:, xi, :, 0:DH])
                load_rest()
            if l == 1:
                nc.gpsimd.collective_compute(
                    "AllGather", mybir.AluOpType.bypass,
                    replica_groups=[list(range(NCORES))],
                    ins=[cc_in[:, :].opt()],
                    outs=[cc_gat[2:NCORES + 2].opt()])
                # band tiles 0..7 from chunks pid-2, pid-1 (slots pid, pid+1)
                for dt_ in range(2):
                    sync.dma_start(
                        out=kT_win[:, dt_, :].rearrange(
                            "p (c x) -> p c x", c=2),
                        in_=cc_gat[bass.ds(pid, 2), :,
                                   dt_ * 512:(dt_ + 1) * 512].rearrange(
                                       "c p x -> p c x"))
                sync.dma_start(
                    out=v_win[:, :, :, :].rearrange(
                        "p (c t) h x -> p c (t h x)", c=2),
                    in_=cc_gat[bass.ds(pid, 2), :, KCH:KCH + VCH].rearrange(
                        "c p x -> p c x"))
                # corner tiles 29, 31 from slot 9 (chunk 7): chunk-tiles 1, 3
                for xi, ct in enumerate((1, 3)):
                    for dt_ in range(2):
                        sync.dma_start(
                            out=kTx[:, dt_, xi * 128:(xi + 1) * 128],
                            in_=cc_gat[9, :, dt_ * 512 + ct * 128:
                                       dt_ * 512 + (ct + 1) * 128])
                    sync.dma_start(
                        out=vx[:, xi, :, :].rearrange("p h x -> p (h x)"),
                        in_=cc_gat[9, :, KCH + ct * H * (DH + 1):
                                   KCH + (ct + 1) * H * (DH + 1)])

            for hh in range(H):
                ph = phs[hh]
                for g0, g1, goff, gcols in GROUPS[2:]:
                    s_group(hh, g0, g1, goff, gcols, ph)
                aoTp = ps_aoT.tile([128, 512], f32, tag="aoT")
                for ri, (w, ts, av_s, av_e) in enumerate(RUNS):
                    ncol = len(ts) * 128
                    q0 = ts[0] * 128
                    if w == "loc4":
                        vv = v4[:, hh, :]
                    elif w >= 8:
                        vv = v_loc[:, w - 8, hh, :]
                    else:
                        vv = v_win[:, w, hh, :]
                    nc.tensor.matmul(
                        aoTp[0:65, q0:q0 + ncol], lhsT=vv,
                        rhs=ph[:, RUNCOLS[ri]:RUNCOLS[ri] + ncol],
                        start=av_s, stop=av_e)
                nc.scalar.copy(out=aoT_sb[0:65, hh, :], in_=aoTp[0:65, :])
                for t in range(4):
                    sync.dma_start_transpose(
                        out=den_all[:, t, hh, :],
                        in_=aoT_sb[64:80, hh, t * 128:(t + 1) * 128])
            # ---- attention, tile 4 (BOS/EOS) : old dense path ----
            for t in (4,):
                kts = _kts_for_tile(t)
                nkt = len(kts)
                nc_cols = nkt * 128
                ao_ps = ps_pr.tile([128, 512], f32, tag="pr")
                for hh in range(H):
                    pb, dt_ = (hh % 2) * 64, hh // 2
                    st = ps_s.tile([128, 512], f32, tag="st")
                    for ki, (kind, w) in enumerate(kts):
                        if kind == "loc4":
                            lhsT = kT4[pb:pb + 64, dt_, :]
                        else:
                            lhsT = kTx[pb:pb + 64, dt_, w * 128:(w + 1) * 128]
                        nc.tensor.matmul(st[:, ki * 128:(ki + 1) * 128],
                                         lhsT=lhsT,
                                         rhs=qT[pb:pb + 64, dt_,
                                                t * 128:(t + 1) * 128],
                                         start=True, stop=True)
                    pe = pe_pool.tile([128, 512], bf16, tag="pe")
                    nc.scalar.activation(out=pe[:, 0:nc_cols], in_=st[:, 0:nc_cols],
                                         func=AF.Exp, scale=float(SCALE))
                    pm = pm_pool.tile([128, 512], bf16, tag="pm")
                    nc.vector.tensor_mul(pm[:, 0:nc_cols], pe[:, 0:nc_cols],
                                         expB4[:, hh, :])
                    for ki, (kind, w) in enumerate(kts):
                        rhs = v4[:, hh, :] if kind == "loc4" else vx[:, w, hh, :]
                        nc.tensor.matmul(ao_ps[:, hh * 65:hh * 65 + 65],
                                         lhsT=pm[:, ki * 128:(ki + 1) * 128],
                                         rhs=rhs, start=(ki == 0),
                                         stop=(ki == nkt - 1))
                rec = wk2_pool.tile([128, 4], f32, tag="rec")
                nc.vector.reciprocal(out=rec[:], in_=ao_ps[:, DH:260:DH + 1])
                ao_sb = wk2_pool.tile([128, D], bf16, tag="aosb")
                for hh in range(H):
                    nc.vector.tensor_scalar(
                        out=ao_sb[:, hh * DH:(hh + 1) * DH],
                        in0=ao_ps[:, hh * 65:hh * 65 + DH],
                        scalar1=rec[:, hh:hh + 1], scalar2=None,
                        op0=mybir.AluOpType.mult)
                aoT = wk2_pool.tile([128, 2, 128], bf16, tag="aoT")
                for dt_ in range(2):
                    sync.dma_start_transpose(
                        out=aoT[:, dt_, :],
                        in_=ao_sb[:, dt_ * 128:(dt_ + 1) * 128])
                xo = ps_pr.tile([128, 512], f32, tag="pr")
                for i in range(2):
                    nc.tensor.matmul(xo[:, 0:D], lhsT=aoT[:, i, :],
                                     rhs=wo_sb[:, i, :],
                                     start=(i == 0), stop=(i == 1))
                nc.vector.tensor_add(x_sb[:, t, :], x_sb[:, t, :], xo[:, 0:D])

            # wo per head; normalization folded into the residual update
            for t in range(4):
                rec = wk2_pool.tile([128, 4], f32, tag="rec")
                nc.vector.reciprocal(out=rec[:], in_=den_all[:, t, :, 0])
                for hh in range(H):
                    xo = ps_pr.tile([128, 512], f32, tag="pr")
                    nc.tensor.matmul(xo[:, 0:D],
                                     lhsT=aoT_sb[0:64, hh, t * 128:(t + 1) * 128],
                                     rhs=woh[:, hh, :], start=True, stop=True)
                    nc.vector.scalar_tensor_tensor(
                        out=x_sb[:, t, :], in0=xo[:, 0:D],
                        scalar=rec[:, hh:hh + 1], in1=x_sb[:, t, :],
                        op0=mybir.AluOpType.mult, op1=mybir.AluOpType.add)

            # ---- FFN ----
            layer_norm([(x_sb[:, t, :], h_sb[:, t, :]) for t in range(NT)],
                       tr=lambda i: transpose_h(h_sb[:, i, :], i * 128))
            w1sb, w2sb = W[("w1", l)], W[("w2", l)]
            for fj in range(8):
                for c0, c1 in ((0, 512), (512, 640)):
                    ps = ps_pr.tile([128, 512], f32, tag="pr")
                    for i in range(2):
                        nc.tensor.matmul(ps[:, 0:c1 - c0],
                                         lhsT=w1sb[:, i, fj * 128:(fj + 1) * 128],
                                         rhs=hT[:, i, c0:c1],
                                         start=(i == 0), stop=(i == 1))
                    nc.scalar.activation(out=yT[:, fj, c0:c1], in_=ps[:, 0:c1 - c0],
                                         func=AF.Gelu, scale=1.0)
            for lt in range(NT):
                ps2 = ps_s.tile([128, 512], f32, tag="st")
                for fj in range(8):
                    nc.tensor.matmul(ps2[:, 0:D],
                                     lhsT=yT[:, fj, lt * 128:(lt + 1) * 128],
                                     rhs=w2sb[:, fj, :],
                                     start=(fj == 0), stop=(fj == 7))
                nc.vector.tensor_add(x_sb[:, lt, :], x_sb[:, lt, :], ps2[:, 0:D])

        # ---------------- final LN + output ----------------
        layer_norm([(x_sb[:, t, :], o_sb[:, t, :]) for t in range(NT)])
        for lt in range(NT):
            sync.dma_start(out=out_d[lt * 128:(lt + 1) * 128, :], in_=o_sb[:, lt, :])

    nc.finalize()
    return nc


# ======================= host side =======================

def _patchify(img):
    x = img.reshape(1, 1, GT, PATCH, GT, PATCH, GT, PATCH)
    x = np.einsum("nctphqwr->nthwpqrc", x).reshape(N, PVOL)
    return np.ascontiguousarray(x).astype(np.float32)


def _host_prep(inputs):
    idx = np.asarray(inputs["idx"])
    valid = np.asarray(inputs["valid"])
    geo = np.asarray(inputs["geo_dist"]).astype(np.float32)
    decay = np.asarray(inputs["decay"]).astype(np.float32)
    K = idx.shape[1]
    fv = valid & (idx <= np.arange(L)[:, None])
    bias_lk = geo[None] * decay[:, None, None]          # [H, L, K]

    patches = _patchify(np.asarray(inputs["input_image"]))
    ids = np.asarray(inputs["input_ids"]).reshape(-1)
    et = np.asarray(inputs["embed_tokens"])
    pb = np.asarray(inputs["patch_b"]).astype(np.float32)
    bos_e, eos_e = et[ids[0]], et[ids[-1]]

    emb = np.zeros((LLOC, D), np.float32)
    emb[0:LC] = pb[None, :]
    emb[LC] = bos_e
    emb[LC + 1] = eos_e

    imgTx = np.concatenate([patches[3712:3840].T, patches[3968:4096].T],
                           axis=1).astype(BF16)          # [64, 256]

    # expB4 (BOS/EOS/pad queries) - same on every core
    expB4 = np.zeros((H, 128, 3 * 128), np.float32)
    expB4[:, 0, 2:128] = 1.0                             # pad queries attend BOS
    for li, gq in ((0, 0), (1, L - 1)):
        for k in range(K):
            if not fv[gq, k]:
                continue
            kr = int(idx[gq, k])
            ev = np.exp(bias_lk[:, gq, k])
            if kr == 0:
                expB4[:, 0, li] = ev
            elif kr == L - 1:
                expB4[:, 1, li] = ev
            else:
                p = kr - 1
                if 3712 <= p < 3840:
                    expB4[:, p - 3712, 1 * 128 + li] = ev
                elif 3968 <= p < 4096:
                    expB4[:, p - 3968, 2 * 128 + li] = ev
                else:
                    raise AssertionError((gq, kr))
    expB4 = expB4.astype(BF16)

    per_core = []
    for c in range(NCORES):
        imgT = np.zeros((PVOL, NBT * 128), np.float32)
        lo = c * LC - 1024
        s0, s1 = max(0, -lo), min(NBT * 128, N - lo)
        imgT[:, s0:s1] = patches[lo + s0:lo + s1].T

        expA = np.zeros((4, H, 128, 8 * 128), np.float32)
        base = c * LC - 1024
        for lq in range(LC):
            gq = 1 + c * LC + lq
            t, lcol = lq // 128, lq % 128
            kts = [t, t + 3, t + 4, t + 5, t + 6, t + 7, t + 8]
            for k in range(K):
                if not fv[gq, k]:
                    continue
                kr = int(idx[gq, k])
                ev = np.exp(bias_lk[:, gq, k])
                if kr == 0:
                    expA[t, :, 0, 7 * 128 + lcol] = ev
                    continue
                wp = kr - 1 - base
                assert 0 <= wp < 1536, (c, gq, kr)
                w, j = wp // 128, wp % 128
                ki = kts.index(w)
                expA[t, :, j, ki * 128 + lcol] = ev
        # regroup per-(t,ki) blocks into the w-grouped run layout
        expW = np.zeros((H, 128, EXPW_COLS), np.float32)
        for ri, (w, ts, _a, _b) in enumerate(RUNS):
            co = RUNCOLS[ri]
            for t in ts:
                ki = 7 if w == "loc4" else [t, t + 3, t + 4, t + 5, t + 6,
                                            t + 7, t + 8].index(w)
                expW[:, :, co + (t - ts[0]) * 128:co + (t - ts[0] + 1) * 128] = \
                    expA[t, :, :, ki * 128:(ki + 1) * 128]
        per_core.append({"imgT": imgT.astype(BF16),
                         "expW": expW.astype(BF16)})

    shared = {
        "imgTx": imgTx,
        "emb": emb,
        "expB4": expB4,
        "patch_w": np.asarray(inputs["patch_w"]).astype(BF16),
    }
    for nm in ("wq", "wk", "wv", "wo", "w1", "w2"):
        shared[nm] = np.asarray(inputs[nm]).astype(BF16)

    # this model instance has trivial LN affine and zero residual biases
    for nm, s_, b_ in (("ln1", inputs["ln1_s"], inputs["ln1_b"]),
                       ("ln2", inputs["ln2_s"], inputs["ln2_b"]),
                       ("lnf", inputs["norm_s"], inputs["norm_b"])):
        assert np.all(np.asarray(s_) == 1.0) and np.all(np.asarray(b_) == 0.0), \
            f"{nm} affine unsupported"
    for nm in ("bo", "b1", "b2"):
        assert np.all(np.asarray(inputs[nm]) == 0.0), f"{nm} nonzero unsupported"

    return per_core, shared


def kernel(**inputs):
    from concourse.bass_utils import run_bass_kernel_spmd

    per_core, shared = _host_prep(inputs)
    if "prog" not in _prog_cache:
        _prog_cache["prog"] = _build_program()
    nc = _prog_cache["prog"]

    in_maps = []
    for c in range(NCORES):
        m = dict(shared)
        m.update(per_core[c])
        in_maps.append(m)
    import os
    trace = bool(os.environ.get("KERNEL_TRACE"))
    res = run_bass_kernel_spmd(nc, in_maps, core_ids=list(range(NCORES)),
                               trace=trace)
    global _last_exec_ns
    _last_exec_ns = res.exec_time_ns

    out = np.zeros((L, D), np.float32)
    for c in range(NCORES):
        out[1 + c * LC:1 + (c + 1) * LC] = res.results[c]["out"][0:LC]
    out[0] = res.results[0]["out"][LC]
    out[L - 1] = res.results[0]["out"][LC + 1]
    return out.reshape(1, L, D)

